# revision 1
# baseline (speedup 1.0000x reference)
"""Bayesian linear layer on 8 TRN2 NeuronCores.

Computes  out = x @ (mu + softplus(rho) * eps_w).T + (bmu + softplus(brho) * eps_b)
for x [16384, 4096], weights [4096, 4096].

Sharding: 2-way split of the batch dim (N) x 4-way split of out_features.
Each core computes an [8192, 1024] fp32 output shard:
  - weight shard W^T is generated on-device: softplus via Exp + Ln(x+1) on the
    ACT engine (table sets batched), FMA on DVE with fp16 output, staged
    through a DRAM scratch, then xbar transpose-loaded as 64 resident tiles
    [128 (in_f), 512 (out_f)] fp16.
  - x is shipped fp16 and xbar transpose-loaded straight from DRAM in
    [1024 x 128] panels (4 k-quarter tiles per 1024-row super-tile).
  - matmuls are fp16, N=512, fp32 PSUM accumulation over 32 k-blocks; the two
    output halves (q=0/1) run as separate phases over 8 PSUM banks so the
    first phase only needs half the prepped weights; bias is added during the
    PSUM->SBUF copy on DVE.
All DMAs stay on the SP HWDGE ring: splitting across the SP+ACT rings
corrupts results on this stack (completion tracking assumes one ring).
"""

import numpy as np

import bass_rust as _bass_rust
import concourse.bacc as bacc
import concourse.tile as tile
from concourse import mybir
from concourse import bass_utils
from concourse.hw_specs import get_activation_tables


class _Bacc(bacc.Bacc):
    """Bacc whose activation-table placement resolves Exp and Ln to the one
    table set containing both (natural_log_exp_and_others), instead of
    thrashing between per-function sets (one 1.3us ACT_TABLE_LOAD per
    ACTIVATE).  List order/indices are preserved -- act_func_set_id is the
    index into act_info.json -- only the membership used for matching is
    restricted."""

    def insert_act_table_loads(self):
        tables = list(get_activation_tables(self.m.arch).items())
        AF = mybir.ActivationFunctionType
        filtered = []
        for name, funcs in tables:
            if name != "natural_log_exp_and_others":
                funcs = funcs - {AF.Exp, AF.Ln}
            filtered.append((name, funcs))
        _bass_rust.insert_act_table_loads(self, filtered)

R, C = 2, 4                      # grid: R-way split of N, C-way split of out_f
N, IN_F, OUT_F = 16384, 4096, 4096
NS, OS = N // R, OUT_F // C      # per-core shards: 8192 rows, 1024 out cols
KB = IN_F // 128                 # 32 k-blocks
NB = 1024                       # rows per super-tile
NKQ = 4                          # k-quarters per super-tile
KQ = KB // NKQ                   # 8 k-blocks per quarter
N_CORES = 8

FP32 = mybir.dt.float32
F16 = mybir.dt.float16


def _build_nc():
    nc = _Bacc("TRN2", target_bir_lowering=False, debug=False)

    xb = nc.dram_tensor("xb", [NS, IN_F], F16, kind="ExternalInput").ap()
    mu = nc.dram_tensor("mu", [OS, IN_F], F16, kind="ExternalInput").ap()
    rho = nc.dram_tensor("rho", [OS, IN_F], F16, kind="ExternalInput").ap()
    eps = nc.dram_tensor("eps", [OS, IN_F], F16, kind="ExternalInput").ap()
    bmu = nc.dram_tensor("bmu", [128, OS], FP32, kind="ExternalInput").ap()
    brho = nc.dram_tensor("brho", [128, OS], FP32, kind="ExternalInput").ap()
    beps = nc.dram_tensor("beps", [128, OS], FP32, kind="ExternalInput").ap()
    out = nc.dram_tensor("out", [NS, OS], FP32, kind="ExternalOutput").ap()

    AF = mybir.ActivationFunctionType
    n_super = NS // NB
    subs = NB // 128

    with tile.TileContext(nc) as tc:
        with (
            tc.tile_pool(name="wt", bufs=1) as wt_pool,
            tc.tile_pool(name="bias", bufs=1) as bias_pool,
            tc.tile_pool(name="prep_rho", bufs=2) as prep_rho,
            tc.tile_pool(name="prep_in", bufs=2) as prep_in,
            tc.tile_pool(name="prep_w", bufs=2) as prep_w,
            tc.tile_pool(name="w16", bufs=1, space="DRAM") as w16_pool,
            tc.tile_pool(name="xt", bufs=1) as xt_pool,
            tc.tile_pool(name="outp", bufs=3) as out_pool,
            tc.tile_pool(name="psum", bufs=1, space="PSUM") as psum_pool,
        ):
            # ---- bias: b = bmu + softplus(brho) * beps, replicated [128, OS]
            bmu_t = bias_pool.tile([128, OS], FP32, tag="bmu")
            brho_t = bias_pool.tile([128, OS], FP32, tag="brho")
            beps_t = bias_pool.tile([128, OS], FP32, tag="beps")
            nc.sync.dma_start(bmu_t[:], bmu[:])
            nc.sync.dma_start(brho_t[:], brho[:])
            nc.sync.dma_start(beps_t[:], beps[:])
            nc.scalar.activation(brho_t[:], brho_t[:], AF.Exp)
            nc.scalar.activation(brho_t[:], brho_t[:], AF.Ln, bias=1.0)
            nc.vector.tensor_mul(beps_t[:], brho_t[:], beps_t[:])
            bias_t = bias_pool.tile([128, OS], FP32, tag="bias")
            nc.vector.tensor_add(bias_t[:], beps_t[:], bmu_t[:])

            # ---- W^T: computed in [o, i] layout, staged to a DRAM scratch
            # (one tile per i-chunk), then transpose-loaded into 32 resident
            # [128, 1024] tiles as each i-chunk completes.
            wts = [wt_pool.tile([128, OS], F16, tag=f"wt{ib}",
                                name=f"wt{ib}") for ib in range(KB)]

            IC = 1024
            NIC = IN_F // IC
            w16 = w16_pool.tile([OS, IN_F], F16, tag="w16", name="w16")

            def prep_group(obs, ic):
                # 4 o-blocks per group: rho loads issued 4-ahead, softplus in
                # place (Exp then Ln(x+1); one table set), then per block
                # w = mu + sp*eps -> fp16, stored to the DRAM scratch.
                rcs = []
                for ob in obs:
                    rho_c = prep_rho.tile([128, IC], F16, tag=f"rho{ob % 4}",
                                          name=f"rho_{ob}_{ic}")
                    nc.sync.dma_start(
                        rho_c[:], rho[ob * 128:(ob + 1) * 128,
                                      ic * IC:(ic + 1) * IC])
                    rcs.append(rho_c)
                for rho_c in rcs:
                    nc.scalar.activation(rho_c[:], rho_c[:], AF.Exp)
                for rho_c in rcs:
                    nc.scalar.activation(rho_c[:], rho_c[:], AF.Ln, bias=1.0)
                for ob, rho_c in zip(obs, rcs):
                    sl = (slice(ob * 128, (ob + 1) * 128),
                          slice(ic * IC, (ic + 1) * IC))
                    mu_c = prep_in.tile([128, IC], F16, tag="mu")
                    eps_c = prep_in.tile([128, IC], F16, tag="eps")
                    nc.sync.dma_start(mu_c[:], mu[sl])
                    nc.sync.dma_start(eps_c[:], eps[sl])
                    t32 = prep_w.tile([128, IC], FP32, tag="t32", bufs=1)
                    nc.vector.tensor_mul(t32[:], rho_c[:], eps_c[:])
                    wf = prep_w.tile([128, IC], F16, tag="wf")
                    nc.vector.tensor_add(wf[:], t32[:], mu_c[:])
                    nc.sync.dma_start(
                        w16[ob * 128:(ob + 1) * 128, ic * IC:(ic + 1) * IC],
                        wf[:])

            def xt_panel(s, kq):
                xtt = xt_pool.tile([128, KQ * NB], F16, tag=f"kq{kq}",
                                   name=f"xt_s{s}_k{kq}",
                                   bufs=2 if kq == 0 else 1)
                for j in range(KQ):
                    ib = kq * KQ + j
                    nc.sync.dma_start(
                        xtt[:, j * NB:(j + 1) * NB],
                        xb[s * NB:(s + 1) * NB, ib * 128:(ib + 1) * 128],
                        transpose=True)
                return xtt

            def xt_panels(s):
                return [xt_panel(s, kq) for kq in range(NKQ)]

            # emission order: all weight prep (grouped), then the first
            # super-tile's x panels, then the 32 big weight transpose-loads.
            # The single w16 tile holds every wts until prep completes: the
            # PE starts once (~165us), stays dense, and never re-throttles;
            # the streaming wtr ops stay just ahead of the PE's k-bursts.
            for ic in range(NIC):
                for g in range(2):
                    prep_group(range(4 * g, 4 * g + 4), ic)
            xtq0 = xt_panels(0)
            for ib in range(KB):
                nc.sync.dma_start(wts[ib][:],
                                  w16[:, ib * 128:(ib + 1) * 128],
                                  transpose=True)

            # ---- main loop
            for s in range(n_super):
                xtq = xtq0 if s == 0 else xt_panels(s)
                for q in range(2):
                    psq = [psum_pool.tile([128, 512], FP32, tag=f"ps{sub}",
                                          name=f"ps_{s}_{q}_{sub}")
                           for sub in range(subs)]
                    for kq in range(NKQ):
                        for sub in range(subs):
                            for j in range(KQ):
                                ib = kq * KQ + j
                                xs = xtq[kq][:, j * NB + sub * 128:
                                             j * NB + (sub + 1) * 128]
                                nc.tensor.matmul(
                                    psq[sub][:], xs,
                                    wts[ib][:, q * 512:(q + 1) * 512],
                                    start=(ib == 0), stop=(ib == KB - 1))
                    for sub in range(subs):
                        ot = out_pool.tile([128, 512], FP32, tag="ot",
                                           name=f"ot_{s}_{q}_{sub}")
                        nc.vector.tensor_add(
                            ot[:], psq[sub][:], bias_t[:, q * 512:(q + 1) * 512])
                        row = (s * subs + sub) * 128
                        nc.sync.dma_start(
                            out[row:row + 128, q * 512:(q + 1) * 512], ot[:])

    nc.compile()
    return nc


_NC = None


def _get_nc():
    global _NC
    if _NC is None:
        _NC = _build_nc()
    return _NC


def kernel(x, weight_mu, weight_rho, bias_mu, bias_rho, eps_w, eps_b,
           _trace=False, _trace_kwargs=None):
    x = np.asarray(x, dtype=np.float32)
    weight_mu = np.asarray(weight_mu, dtype=np.float32)
    weight_rho = np.asarray(weight_rho, dtype=np.float32)
    bias_mu = np.asarray(bias_mu, dtype=np.float32)
    bias_rho = np.asarray(bias_rho, dtype=np.float32)
    eps_w = np.asarray(eps_w, dtype=np.float32)
    eps_b = np.asarray(eps_b, dtype=np.float32)

    nc = _get_nc()
    xb = x.astype(np.float16)

    in_maps = []
    for c in range(N_CORES):
        r, q = divmod(c, C)
        osl = slice(q * OS, (q + 1) * OS)
        in_maps.append({
            "xb": xb[r * NS:(r + 1) * NS],
            "mu": weight_mu[osl].astype(np.float16),
            "rho": weight_rho[osl].astype(np.float16),
            "eps": eps_w[osl].astype(np.float16),
            "bmu": np.ascontiguousarray(np.broadcast_to(bias_mu[osl], (128, OS))),
            "brho": np.ascontiguousarray(np.broadcast_to(bias_rho[osl], (128, OS))),
            "beps": np.ascontiguousarray(np.broadcast_to(eps_b[osl], (128, OS))),
        })

    kwargs = {}
    if _trace:
        kwargs["trace"] = True
        if _trace_kwargs:
            kwargs.update(_trace_kwargs)
    res = bass_utils.run_bass_kernel_spmd(
        nc, in_maps, core_ids=list(range(N_CORES)), **kwargs)

    out = np.empty((N, OUT_F), np.float32)
    for c in range(N_CORES):
        r, q = divmod(c, C)
        out[r * NS:(r + 1) * NS, q * OS:(q + 1) * OS] = res.results[c]["out"]
    if _trace:
        return out, res
    return out



# revision 2
# speedup vs baseline: 1.2373x; 1.2373x over previous
"""Bayesian linear layer on 8 TRN2 NeuronCores.

Computes  out = x @ (mu + softplus(rho) * eps_w).T + (bmu + softplus(brho) * eps_b)
for x [16384, 4096], weights [4096, 4096].

Sharding: 2-way split of the batch dim (N) x 4-way split of out_features.
Each core computes an [8192, 1024] fp32 output shard.

Design notes (v2):
  - Weight inputs are shipped host-transposed ([in_f, out_f] fp16), so the
    device materializes W^T = mu + softplus(rho)*eps with cheap LINEAR loads
    (0.6us SP dispatch per chunk vs 1.3us for a DMA transpose) and the
    elementwise softplus/FMA run directly in [i, o] layout.  Weights live in
    8 resident quad tiles [128, 4x1024] fp16; softplus is Exp then Ln(x+1)
    on ACT at FD=4096 to amortize the 352-cycle instruction overhead.
  - x is shipped fp16 and xbar transpose-loaded in [512 x 128] chunks into
    double-buffered k-quarter panels (NB=512 row super-tiles).
  - Matmuls are fp16, N=512 moving, fp32 PSUM.  Phase = (super-tile, q-half);
    q=0 phases use PSUM banks 0-3, q=1 banks 4-7, and each bank is drained
    (DVE bias-add) right after its 32-matmul k-chain, so phase transitions
    never wait on banks.  Super-tile 0 instead interleaves both q halves
    across all 8 banks in k-arrival order, so the PE starts consuming weight
    quads ~10us in, overlapping the whole prep stream.
  - bias = bmu + softplus(brho)*eps_b is computed on one partition from
    [1, OS] rows and broadcast to [128, OS] with a K=1 ones-matmul.
All DMAs stay on the SP HWDGE ring: splitting across the SP+ACT rings
corrupts results on this stack (completion tracking assumes one ring).
"""

import numpy as np

import bass_rust as _bass_rust
import concourse.bacc as bacc
import concourse.tile as tile
from concourse import mybir
from concourse import bass_utils
from concourse.hw_specs import get_activation_tables


class _Bacc(bacc.Bacc):
    """Bacc whose activation-table placement resolves Exp and Ln to the one
    table set containing both (natural_log_exp_and_others), instead of
    thrashing between per-function sets (one 1.3us ACT_TABLE_LOAD per
    ACTIVATE)."""

    def insert_act_table_loads(self):
        tables = list(get_activation_tables(self.m.arch).items())
        AF = mybir.ActivationFunctionType
        filtered = []
        for name, funcs in tables:
            if name != "natural_log_exp_and_others":
                funcs = funcs - {AF.Exp, AF.Ln}
            filtered.append((name, funcs))
        _bass_rust.insert_act_table_loads(self, filtered)


R, C = 2, 4                      # grid: R-way split of N, C-way split of out_f
N, IN_F, OUT_F = 16384, 4096, 4096
NS, OS = N // R, OUT_F // C      # per-core shards: 8192 rows, 1024 out cols
KB = IN_F // 128                 # 32 k-blocks
NB = 512                         # rows per super-tile
SUBS = NB // 128                 # 4 row-subtiles per super-tile
NSUP = NS // NB                  # 16 super-tiles
NKQ = 4                          # k-quarters per super-tile panel set
KQ = KB // NKQ                   # 8 k-blocks per quarter
QUAD = 4                         # k-blocks per weight quad tile
NQUADS = KB // QUAD              # 8 weight quad tiles
N_CORES = 8

FP32 = mybir.dt.float32
F16 = mybir.dt.float16


def _build_nc():
    nc = _Bacc("TRN2", target_bir_lowering=False, debug=False)

    xb = nc.dram_tensor("xb", [NS, IN_F], F16, kind="ExternalInput").ap()
    # host-transposed weight params: [in_f, out_f] for this core's o-shard
    muT = nc.dram_tensor("muT", [IN_F, OS], F16, kind="ExternalInput").ap()
    rhoT = nc.dram_tensor("rhoT", [IN_F, OS], F16, kind="ExternalInput").ap()
    epsT = nc.dram_tensor("epsT", [IN_F, OS], F16, kind="ExternalInput").ap()
    bmu = nc.dram_tensor("bmu", [1, OS], FP32, kind="ExternalInput").ap()
    brho = nc.dram_tensor("brho", [1, OS], FP32, kind="ExternalInput").ap()
    beps = nc.dram_tensor("beps", [1, OS], FP32, kind="ExternalInput").ap()
    ones = nc.dram_tensor("ones", [1, 128], FP32, kind="ExternalInput").ap()
    out = nc.dram_tensor("out", [NS, OS], FP32, kind="ExternalOutput").ap()

    AF = mybir.ActivationFunctionType

    with tile.TileContext(nc) as tc:
        with (
            tc.tile_pool(name="wt", bufs=1) as wt_pool,
            tc.tile_pool(name="bias", bufs=1) as bias_pool,
            tc.tile_pool(name="stage", bufs=2) as stage_pool,
            tc.tile_pool(name="xt", bufs=2) as xt_pool,
            tc.tile_pool(name="outp", bufs=4) as out_pool,
            tc.tile_pool(name="psum", bufs=1, space="PSUM") as psum_pool,
        ):
            # ---- 8 psum accumulators: tag (q, sub) -> one bank each
            def ps_tile(q, sub, s):
                return psum_pool.tile([128, 512], FP32, tag=f"ps{q}{sub}",
                                      name=f"ps_{s}_{q}_{sub}")

            # ---- bias: row [1, OS] then ones-matmul broadcast to [128, OS]
            ones_t = bias_pool.tile([1, 128], FP32, tag="ones")
            bmu_r = bias_pool.tile([1, OS], FP32, tag="bmu")
            brho_r = bias_pool.tile([1, OS], FP32, tag="brho")
            beps_r = bias_pool.tile([1, OS], FP32, tag="beps")
            nc.sync.dma_start(ones_t[:], ones[:])
            nc.sync.dma_start(bmu_r[:], bmu[:])
            nc.sync.dma_start(brho_r[:], brho[:])
            nc.sync.dma_start(beps_r[:], beps[:])
            nc.scalar.activation(brho_r[:], brho_r[:], AF.Exp)
            nc.scalar.activation(brho_r[:], brho_r[:], AF.Ln, bias=1.0)
            nc.vector.tensor_mul(beps_r[:], brho_r[:], beps_r[:])
            nc.vector.tensor_add(bmu_r[:], beps_r[:], bmu_r[:])
            bias_t = bias_pool.tile([128, OS], FP32, tag="bias")
            for q in range(2):
                bps = ps_tile(1, 2 + q, -1)     # borrow q1 banks; done early
                nc.tensor.matmul(bps[:], ones_t[:], bmu_r[:, q*512:(q+1)*512],
                                 start=True, stop=True)
                nc.vector.tensor_copy(bias_t[:, q*512:(q+1)*512], bps[:])

            # ---- x panels: per super-tile, 4 k-quarter panels of 8 chunks
            def xt_panels(s):
                panels = []
                for kq in range(NKQ):
                    xtt = xt_pool.tile([128, KQ * NB], F16, tag=f"kq{kq}",
                                       name=f"xt_s{s}_k{kq}")
                    for j in range(KQ):
                        ib = kq * KQ + j
                        nc.sync.dma_start(
                            xtt[:, j * NB:(j + 1) * NB],
                            xb[s * NB:(s + 1) * NB, ib * 128:(ib + 1) * 128],
                            transpose=True)
                    panels.append(xtt)
                return panels

            def xs_slice(panels, ib, sub):
                kq, j = divmod(ib, KQ)
                return panels[kq][:, j * NB + sub * 128:
                                  j * NB + (sub + 1) * 128]

            # ---- weight quads: wts[g][:, (ib%4)*1024 + o] for ib in quad g
            wts = [wt_pool.tile([128, QUAD * OS], F16, tag=f"wt{g}",
                                name=f"wt{g}") for g in range(NQUADS)]

            def w_slice(ib, q):
                g, jj = divmod(ib, QUAD)
                return wts[g][:, jj * OS + q * 512: jj * OS + (q + 1) * 512]

            xtq0 = xt_panels(0)

            def prep_quad(g):
                rho_s = stage_pool.tile([128, QUAD * OS], F16, tag="rho",
                                        name=f"rho{g}")
                mu_s = stage_pool.tile([128, QUAD * OS], F16, tag="mu",
                                       name=f"mu{g}")
                eps_s = stage_pool.tile([128, QUAD * OS], F16, tag="eps",
                                        name=f"eps{g}")
                for jj in range(QUAD):
                    ib = g * QUAD + jj
                    sl = slice(ib * 128, (ib + 1) * 128)
                    nc.sync.dma_start(rho_s[:, jj*OS:(jj+1)*OS], rhoT[sl, :])
                for jj in range(QUAD):
                    ib = g * QUAD + jj
                    sl = slice(ib * 128, (ib + 1) * 128)
                    nc.sync.dma_start(mu_s[:, jj*OS:(jj+1)*OS], muT[sl, :])
                    nc.sync.dma_start(eps_s[:, jj*OS:(jj+1)*OS], epsT[sl, :])
                nc.scalar.activation(rho_s[:], rho_s[:], AF.Exp)
                nc.scalar.activation(rho_s[:], rho_s[:], AF.Ln, bias=1.0)
                nc.vector.tensor_mul(eps_s[:], rho_s[:], eps_s[:])
                nc.vector.tensor_add(wts[g][:], eps_s[:], mu_s[:])

            for g in range(NQUADS):
                prep_quad(g)

            # ---- super-tile 0: both q halves, all 8 banks, k-arrival order
            ps0 = {(q, sub): ps_tile(q, sub, 0)
                   for q in range(2) for sub in range(SUBS)}
            for ib in range(KB):
                for q in range(2):
                    for sub in range(SUBS):
                        nc.tensor.matmul(
                            ps0[(q, sub)][:], xs_slice(xtq0, ib, sub),
                            w_slice(ib, q),
                            start=(ib == 0), stop=(ib == KB - 1))

            def drain(ps, s, q, sub):
                ot = out_pool.tile([128, 512], FP32, tag="ot",
                                   name=f"ot_{s}_{q}_{sub}")
                nc.vector.tensor_add(ot[:], ps[:],
                                     bias_t[:, q * 512:(q + 1) * 512])
                row = s * NB + sub * 128
                nc.sync.dma_start(out[row:row + 128, q*512:(q+1)*512], ot[:])

            for q in range(2):
                for sub in range(SUBS):
                    drain(ps0[(q, sub)], 0, q, sub)

            # ---- super-tiles 1..: sub-outer phases, per-bank early drains
            panels = xt_panels(1)
            for s in range(1, NSUP):
                nxt = xt_panels(s + 1) if s + 1 < NSUP else None
                for q in range(2):
                    for sub in range(SUBS):
                        ps = ps_tile(q, sub, s)
                        for ib in range(KB):
                            nc.tensor.matmul(
                                ps[:], xs_slice(panels, ib, sub),
                                w_slice(ib, q),
                                start=(ib == 0), stop=(ib == KB - 1))
                        drain(ps, s, q, sub)
                panels = nxt

    nc.compile()
    return nc


_NC = None


def _get_nc():
    global _NC
    if _NC is None:
        _NC = _build_nc()
    return _NC


def kernel(x, weight_mu, weight_rho, bias_mu, bias_rho, eps_w, eps_b,
           _trace=False, _trace_kwargs=None):
    x = np.asarray(x, dtype=np.float32)
    weight_mu = np.asarray(weight_mu, dtype=np.float32)
    weight_rho = np.asarray(weight_rho, dtype=np.float32)
    bias_mu = np.asarray(bias_mu, dtype=np.float32)
    bias_rho = np.asarray(bias_rho, dtype=np.float32)
    eps_w = np.asarray(eps_w, dtype=np.float32)
    eps_b = np.asarray(eps_b, dtype=np.float32)

    nc = _get_nc()
    xb = x.astype(np.float16)
    muT = np.ascontiguousarray(weight_mu.T).astype(np.float16)
    rhoT = np.ascontiguousarray(weight_rho.T).astype(np.float16)
    epsT = np.ascontiguousarray(eps_w.T).astype(np.float16)
    ones = np.ones((1, 128), np.float32)

    in_maps = []
    for c in range(N_CORES):
        r, q = divmod(c, C)
        osl = slice(q * OS, (q + 1) * OS)
        in_maps.append({
            "xb": xb[r * NS:(r + 1) * NS],
            "muT": np.ascontiguousarray(muT[:, osl]),
            "rhoT": np.ascontiguousarray(rhoT[:, osl]),
            "epsT": np.ascontiguousarray(epsT[:, osl]),
            "bmu": bias_mu[osl].reshape(1, OS),
            "brho": bias_rho[osl].reshape(1, OS),
            "beps": eps_b[osl].reshape(1, OS),
            "ones": ones,
        })

    kwargs = {}
    if _trace:
        kwargs["trace"] = True
        if _trace_kwargs:
            kwargs.update(_trace_kwargs)
    res = bass_utils.run_bass_kernel_spmd(
        nc, in_maps, core_ids=list(range(N_CORES)), **kwargs)

    out = np.empty((N, OUT_F), np.float32)
    for c in range(N_CORES):
        r, q = divmod(c, C)
        out[r * NS:(r + 1) * NS, q * OS:(q + 1) * OS] = res.results[c]["out"]
    if _trace:
        return out, res
    return out


# revision 8
# speedup vs baseline: 1.3114x; 1.0599x over previous
"""Bayesian linear layer on 8 TRN2 NeuronCores.

Computes  out = x @ (mu + softplus(rho) * eps_w).T + (bmu + softplus(brho) * eps_b)
for x [16384, 4096], weights [4096, 4096].

Sharding: 2-way split of the batch dim (N) x 4-way split of out_features.
Each core computes an [8192, 1024] fp32 output shard.

Design notes (v2):
  - Weight inputs are shipped host-transposed ([in_f, out_f] fp16), so the
    device materializes W^T = mu + softplus(rho)*eps with cheap LINEAR loads
    (0.6us SP dispatch per chunk vs 1.3us for a DMA transpose) and the
    elementwise softplus/FMA run directly in [i, o] layout.  Weights live in
    8 resident quad tiles [128, 4x1024] fp16; softplus is Exp then Ln(x+1)
    on ACT at FD=4096 to amortize the 352-cycle instruction overhead.
  - x is shipped host-transposed ([in_f, n] fp16) so k-major panels load
    with linear [128 x 1KB] DIRECT2D chunks (~0.6us SP dispatch each) into
    double-buffered k-quarter panels (NB=512 row super-tiles).  A DMA
    transpose dispatch costs a fixed ~1.3us of SP time regardless of rows,
    so 512 of them (measured 656us) would starve the SP ring.
  - Matmuls are fp16, N=512 moving, fp32 PSUM.  Phase = (super-tile, q-half);
    q=0 phases use PSUM banks 0-3, q=1 banks 4-7, and each bank is drained
    (DVE bias-add) right after its 32-matmul k-chain, so phase transitions
    never wait on banks.  Super-tile 0 instead interleaves both q halves
    across all 8 banks in k-arrival order, so the PE starts consuming weight
    quads ~10us in, overlapping the whole prep stream.
  - bias = bmu + softplus(brho)*eps_b is computed on one partition from
    [1, OS] rows and broadcast to [128, OS] with a K=1 ones-matmul.
All DMAs stay on the SP HWDGE ring: splitting across the SP+ACT rings
corrupts results on this stack (completion tracking assumes one ring).
"""

import numpy as np

import bass_rust as _bass_rust
import concourse.bacc as bacc
import concourse.tile as tile
from concourse import mybir
from concourse import bass_utils
from concourse.hw_specs import get_activation_tables


class _Bacc(bacc.Bacc):
    """Bacc whose activation-table placement resolves Exp and Ln to the one
    table set containing both (natural_log_exp_and_others), instead of
    thrashing between per-function sets (one 1.3us ACT_TABLE_LOAD per
    ACTIVATE)."""

    def insert_act_table_loads(self):
        tables = list(get_activation_tables(self.m.arch).items())
        AF = mybir.ActivationFunctionType
        filtered = []
        for name, funcs in tables:
            if name != "natural_log_exp_and_others":
                funcs = funcs - {AF.Exp, AF.Ln}
            filtered.append((name, funcs))
        _bass_rust.insert_act_table_loads(self, filtered)


R, C = 2, 4                      # grid: R-way split of N, C-way split of out_f
N, IN_F, OUT_F = 16384, 4096, 4096
NS, OS = N // R, OUT_F // C      # per-core shards: 8192 rows, 1024 out cols
KB = IN_F // 128                 # 32 k-blocks
NB = 512                         # rows per super-tile
SUBS = NB // 128                 # 4 row-subtiles per super-tile
NSUP = NS // NB                  # 16 super-tiles
NKQ = 4                          # k-quarters per super-tile panel set
KQ = KB // NKQ                   # 8 k-blocks per quarter
QUAD = 4                         # k-blocks per weight quad tile
NQUADS = KB // QUAD              # 8 weight quad tiles
N_CORES = 8

FP32 = mybir.dt.float32
F16 = mybir.dt.float16


def _build_nc():
    nc = _Bacc("TRN2", target_bir_lowering=False, debug=False)

    xT = nc.dram_tensor("xT", [IN_F, NS], F16, kind="ExternalInput").ap()
    # host-transposed weight params: [in_f, out_f] for this core's o-shard
    muT = nc.dram_tensor("muT", [IN_F, OS], F16, kind="ExternalInput").ap()
    rhoT = nc.dram_tensor("rhoT", [IN_F, OS], F16, kind="ExternalInput").ap()
    epsT = nc.dram_tensor("epsT", [IN_F, OS], F16, kind="ExternalInput").ap()
    bmu = nc.dram_tensor("bmu", [1, OS], FP32, kind="ExternalInput").ap()
    brho = nc.dram_tensor("brho", [1, OS], FP32, kind="ExternalInput").ap()
    beps = nc.dram_tensor("beps", [1, OS], FP32, kind="ExternalInput").ap()
    ones = nc.dram_tensor("ones", [1, 128], FP32, kind="ExternalInput").ap()
    out = nc.dram_tensor("out", [NS, OS], FP32, kind="ExternalOutput").ap()

    AF = mybir.ActivationFunctionType

    with tile.TileContext(nc) as tc:
        with (
            tc.tile_pool(name="wt", bufs=1) as wt_pool,
            tc.tile_pool(name="bias", bufs=1) as bias_pool,
            tc.tile_pool(name="stage", bufs=2) as stage_pool,
            tc.tile_pool(name="xt", bufs=2) as xt_pool,
            tc.tile_pool(name="outp", bufs=4) as out_pool,
            tc.tile_pool(name="psum", bufs=1, space="PSUM") as psum_pool,
        ):
            # ---- 8 psum accumulators: tag (q, sub) -> one bank each
            def ps_tile(q, sub, s):
                return psum_pool.tile([128, 512], FP32, tag=f"ps{q}{sub}",
                                      name=f"ps_{s}_{q}_{sub}")

            # ---- bias: row [1, OS] then ones-matmul broadcast to [128, OS]
            ones_t = bias_pool.tile([1, 128], FP32, tag="ones")
            bmu_r = bias_pool.tile([1, OS], FP32, tag="bmu")
            brho_r = bias_pool.tile([1, OS], FP32, tag="brho")
            beps_r = bias_pool.tile([1, OS], FP32, tag="beps")
            nc.sync.dma_start(ones_t[:], ones[:])
            nc.sync.dma_start(bmu_r[:], bmu[:])
            nc.sync.dma_start(brho_r[:], brho[:])
            nc.sync.dma_start(beps_r[:], beps[:])
            nc.scalar.activation(brho_r[:], brho_r[:], AF.Exp)
            nc.scalar.activation(brho_r[:], brho_r[:], AF.Ln, bias=1.0)
            nc.vector.tensor_mul(beps_r[:], brho_r[:], beps_r[:])
            nc.vector.tensor_add(bmu_r[:], beps_r[:], bmu_r[:])
            bias_t = bias_pool.tile([128, OS], FP32, tag="bias")
            for q in range(2):
                bps = ps_tile(1, 2 + q, -1)     # borrow q1 banks; done early
                nc.tensor.matmul(bps[:], ones_t[:], bmu_r[:, q*512:(q+1)*512],
                                 start=True, stop=True)
                nc.vector.tensor_copy(bias_t[:, q*512:(q+1)*512], bps[:])

            # ---- x panels: per super-tile, 4 k-quarter panels of 8 chunks
            def xt_panel(s, kq):
                xtt = xt_pool.tile([128, KQ * NB], F16, tag=f"kq{kq}",
                                   name=f"xt_s{s}_k{kq}")
                for j in range(KQ):
                    ib = kq * KQ + j
                    nc.sync.dma_start(
                        xtt[:, j * NB:(j + 1) * NB],
                        xT[ib * 128:(ib + 1) * 128, s * NB:(s + 1) * NB])
                return xtt

            def xt_panels(s):
                return [xt_panel(s, kq) for kq in range(NKQ)]

            def xs_slice(panels, ib, sub):
                kq, j = divmod(ib, KQ)
                return panels[kq][:, j * NB + sub * 128:
                                  j * NB + (sub + 1) * 128]

            # ---- weight quads: wts[g][:, (ib%4)*1024 + o] for ib in quad g
            wts = [wt_pool.tile([128, QUAD * OS], F16, tag=f"wt{g}",
                                name=f"wt{g}") for g in range(NQUADS)]

            def w_slice(ib, q):
                g, jj = divmod(ib, QUAD)
                return wts[g][:, jj * OS + q * 512: jj * OS + (q + 1) * 512]

            def prep_quad(g):
                rho_s = stage_pool.tile([128, QUAD * OS], F16, tag="rho",
                                        name=f"rho{g}")
                mu_s = stage_pool.tile([128, QUAD * OS], F16, tag="mu",
                                       name=f"mu{g}")
                eps_s = stage_pool.tile([128, QUAD * OS], F16, tag="eps",
                                        name=f"eps{g}")
                for jj in range(QUAD):
                    ib = g * QUAD + jj
                    sl = slice(ib * 128, (ib + 1) * 128)
                    nc.sync.dma_start(rho_s[:, jj*OS:(jj+1)*OS], rhoT[sl, :])
                for jj in range(QUAD):
                    ib = g * QUAD + jj
                    sl = slice(ib * 128, (ib + 1) * 128)
                    nc.sync.dma_start(mu_s[:, jj*OS:(jj+1)*OS], muT[sl, :])
                    nc.sync.dma_start(eps_s[:, jj*OS:(jj+1)*OS], epsT[sl, :])
                nc.scalar.activation(rho_s[:], rho_s[:], AF.Exp)
                nc.scalar.activation(rho_s[:], rho_s[:], AF.Ln, bias=1.0)
                nc.vector.tensor_mul(eps_s[:], rho_s[:], eps_s[:])
                nc.vector.tensor_add(wts[g][:], eps_s[:], mu_s[:])

            # emission order drives SP dispatch order: quad g's loads first
            # (ACT softplus chain is the startup critical path), s0's x
            # panels interleaved between the first quads.
            xtq0 = []
            for g in range(NQUADS):
                prep_quad(g)
                if g < NKQ:
                    xtq0.append(xt_panel(0, g))

            # ---- super-tile 0: both q halves, all 8 banks, k-arrival order
            ps0 = {(q, sub): ps_tile(q, sub, 0)
                   for q in range(2) for sub in range(SUBS)}
            for ib in range(KB):
                for q in range(2):
                    for sub in range(SUBS):
                        nc.tensor.matmul(
                            ps0[(q, sub)][:], xs_slice(xtq0, ib, sub),
                            w_slice(ib, q),
                            start=(ib == 0), stop=(ib == KB - 1))

            def drain(ps, s, q, sub):
                ot = out_pool.tile([128, 512], FP32, tag="ot",
                                   name=f"ot_{s}_{q}_{sub}")
                nc.vector.tensor_add(ot[:], ps[:],
                                     bias_t[:, q * 512:(q + 1) * 512])
                row = s * NB + sub * 128
                nc.sync.dma_start(out[row:row + 128, q*512:(q+1)*512], ot[:])

            for q in range(2):
                for sub in range(SUBS):
                    drain(ps0[(q, sub)], 0, q, sub)

            # ---- super-tiles 1..: sub-outer phases, per-bank early drains
            panels = xt_panels(1)
            for s in range(1, NSUP):
                nxt = xt_panels(s + 1) if s + 1 < NSUP else None
                for q in range(2):
                    for sub in range(SUBS):
                        ps = ps_tile(q, sub, s)
                        for ib in range(KB):
                            nc.tensor.matmul(
                                ps[:], xs_slice(panels, ib, sub),
                                w_slice(ib, q),
                                start=(ib == 0), stop=(ib == KB - 1))
                        drain(ps, s, q, sub)
                panels = nxt

    nc.compile()
    return nc


_NC = None


def _get_nc():
    global _NC
    if _NC is None:
        _NC = _build_nc()
    return _NC


def kernel(x, weight_mu, weight_rho, bias_mu, bias_rho, eps_w, eps_b,
           _trace=False, _trace_kwargs=None):
    x = np.asarray(x, dtype=np.float32)
    weight_mu = np.asarray(weight_mu, dtype=np.float32)
    weight_rho = np.asarray(weight_rho, dtype=np.float32)
    bias_mu = np.asarray(bias_mu, dtype=np.float32)
    bias_rho = np.asarray(bias_rho, dtype=np.float32)
    eps_w = np.asarray(eps_w, dtype=np.float32)
    eps_b = np.asarray(eps_b, dtype=np.float32)

    nc = _get_nc()
    xT = np.ascontiguousarray(x.T).astype(np.float16)
    muT = np.ascontiguousarray(weight_mu.T).astype(np.float16)
    rhoT = np.ascontiguousarray(weight_rho.T).astype(np.float16)
    epsT = np.ascontiguousarray(eps_w.T).astype(np.float16)
    ones = np.ones((1, 128), np.float32)

    in_maps = []
    for c in range(N_CORES):
        r, q = divmod(c, C)
        osl = slice(q * OS, (q + 1) * OS)
        in_maps.append({
            "xT": np.ascontiguousarray(xT[:, r * NS:(r + 1) * NS]),
            "muT": np.ascontiguousarray(muT[:, osl]),
            "rhoT": np.ascontiguousarray(rhoT[:, osl]),
            "epsT": np.ascontiguousarray(epsT[:, osl]),
            "bmu": bias_mu[osl].reshape(1, OS),
            "brho": bias_rho[osl].reshape(1, OS),
            "beps": eps_b[osl].reshape(1, OS),
            "ones": ones,
        })

    kwargs = {}
    if _trace:
        kwargs["trace"] = True
        if _trace_kwargs:
            kwargs.update(_trace_kwargs)
    res = bass_utils.run_bass_kernel_spmd(
        nc, in_maps, core_ids=list(range(N_CORES)), **kwargs)

    out = np.empty((N, OUT_F), np.float32)
    for c in range(N_CORES):
        r, q = divmod(c, C)
        out[r * NS:(r + 1) * NS, q * OS:(q + 1) * OS] = res.results[c]["out"]
    if _trace:
        return out, res
    return out
